# revision 6
# baseline (speedup 1.0000x reference)
"""GAT (DGL GATConv) over complete per-doc graphs — Trainium2 Bass kernel, v3.

Problem: nn_CompletedSentenceGraph (gnn_message_passing).
  64 docs x 512 sentences, HIDDEN=256, HEADS=4, D=256.
  h = (x @ W).reshape(B,S,H,D)
  el/er = einsum(h, attn_l/attn_r)
  e[b,s,t,h] = leaky_relu(el[s]+er[t], 0.2); alpha = softmax over s
  out = einsum(alpha, h) + bias; return mean over heads  -> [N, 256]

Sharding: data-parallel over docs, 8 docs per core on 8 cores.

Math tricks (same family as v2):
  * exp(lrelu(x)) = max(exp(x), exp(0.2 x)); with x = el_s + er_t both exps
    are rank-1: expe'[s,t] = max(a_s, c_s * m_t), a=exp(el), c=exp(0.2 el),
    m=exp(-0.8 er) (per-dst scaling; softmax-invariant).
  * el/er from an augmented tiny matmul (WLR = W @ ALR built on host).
  * Z (softmax denominator) via near-free N=1 matmuls with a 4.0-ones rhs;
    4.0 folds the 1/H head-mean.

v3 pipeline changes (driven by TimelineSim engine-occupancy analysis):
  * One shared PSUM bank ("misc") holds the elr results + the er-transpose
    + the Z columns (PSUM slots are bank-padded, so separate tags would
    burn 3 of 8 banks); the freed bank deepens the agg pool to 4 banks.
  * proj ss-chunks and agg dc-chunks interleave on PE so psum drains
    (ACT/DVE) overlap matmuls instead of stalling them.
  * Head-combine rebalanced: DVE does rz + 3 fused scale-adds, ACT does 1
    scaled copy, the final bf16 add runs on the idle Pool engine.
  * 1 output DMA per doc (was 4); the m-row collapse DMA moved to the
    Pool software-DGE queue (off the shared HWDGE + SP queue).
  * Startup DMA order: xt(0), wlr, w, bias, xt(1), xt(2) — gets the first
    elr/proj matmuls their operands ~1us sooner.
"""

from contextlib import ExitStack

import numpy as np

import concourse.mybir as mybir
import concourse.tile as tile
from concourse import bacc
from concourse.bass_utils import run_bass_kernel_spmd
from concourse.masks import make_identity

F32 = mybir.dt.float32
BF16 = mybir.dt.bfloat16
AX = mybir.AluOpType
ACTF = mybir.ActivationFunctionType

NUM_DOCS = 64
S = 512          # sentences per doc
K = 256          # hidden
H = 4            # heads
D = 256          # per-head out feats
N_CORES = 8
DPC = NUM_DOCS // N_CORES  # docs per core
P = 128

SS = S // P      # 4 s-subtiles per doc
KC = K // P      # 2 k-chunks
DC = S // P      # 4 dst chunks

# misc psum bank layout (f32 columns)
MC_ELR = 0       # [P, SS, 8] elr results: cols 0..32
MC_TRM = 32      # [16, 128] er-transpose: cols 32..160 (partitions 0..16)
MC_PZ = 160      # [P, DC, H] Z sums: cols 160..176


def gat_tile_kernel(tc, xt, w, wlr, bias_m, out):
    """xt [DPC, 128, KC, 512] bf16 = host-pre-transposed x;
    w [256, 1024] bf16; wlr [256, 8] bf16 = host [W@attn_r | W@attn_l];
    bias_m [1, 256] bf16 = host mean-over-heads bias."""
    nc = tc.nc

    stack = ExitStack()
    with stack:
        consts = stack.enter_context(tc.tile_pool(name="consts", bufs=1))
        ps_misc = stack.enter_context(
            tc.tile_pool(name="ps_misc", bufs=1, space="PSUM"))
        # setup consts are emitted AFTER the first x load (see below) so the
        # doc-0 x DMA gets the head of the SP queue; setup_tmp stays open for
        # the whole program: closing it would emit a pool-exit barrier that
        # stalls the SP queue.
        setup_tmp = stack.enter_context(tc.tile_pool(name="setup_tmp", bufs=1))
        cst = {}

        def emit_setup():
            ident_f32 = consts.tile([P, P], F32)
            make_identity(nc, ident_f32)

            # weights arrive bf16 from the host: plain DMAs, no converts.
            # wlr FIRST (elr needs it before proj needs w).
            wlr_bf = consts.tile([P, KC, 8], BF16)
            nc.sync.dma_start(out=wlr_bf,
                              in_=wlr.rearrange("(kc p) c -> p kc c", p=P))
            w_bf = consts.tile([P, KC, H * D], BF16)
            nc.sync.dma_start(out=w_bf, in_=w.rearrange("(kc p) f -> p kc f", p=P))
            bias_mf = setup_tmp.tile([1, D], BF16)
            nc.sync.dma_start(out=bias_mf, in_=bias_m)
            bias_b = consts.tile([P, D], BF16)
            nc.gpsimd.partition_broadcast(bias_b, bias_mf)

            # ones column for the Z matmuls; 4.0 folds the 1/H head mean
            ones4 = consts.tile([P, 1], BF16)
            nc.gpsimd.memset(ones4, 4.0)
            cst.update(ident_f32=ident_f32, w_bf=w_bf,
                       wlr_bf=wlr_bf, bias_b=bias_b, ones4=ones4)

        # ---------------- per-doc pipeline ----------------
        with tc.tile_pool(name="xtp", bufs=4) as xtp, \
             tc.tile_pool(name="hp", bufs=3) as hp, \
             tc.tile_pool(name="ep", bufs=3) as ep, \
             tc.tile_pool(name="mp", bufs=3) as mp, \
             tc.tile_pool(name="sp", bufs=4) as sp, \
             tc.tile_pool(name="accp", bufs=3) as accp, \
             tc.tile_pool(name="ps_proj", bufs=3, space="PSUM") as ps_proj, \
             tc.tile_pool(name="ps_agg", bufs=4, space="PSUM") as ps_agg:

            def stage_x(d):
                """xT arrives pre-transposed/pre-bf16 from the host: 1 DMA."""
                xt_bf = xtp.tile([P, KC, S], BF16)
                nc.sync.dma_start(out=xt_bf, in_=xt[d])
                return xt_bf

            def misc_tile():
                """One psum bank shared by elr results / transpose / Z."""
                m = ps_misc.tile([P, 512], F32, tag="misc")
                return {
                    "pcall": m[:, MC_ELR:MC_ELR + SS * 8]
                    .rearrange("p (s c) -> p s c", s=SS),
                    "trm": m[0:4 * SS, MC_TRM:MC_TRM + P],
                    "pz": m[:, MC_PZ:MC_PZ + DC * H]
                    .rearrange("p (d h) -> p d h", d=DC),
                }

            def stage_elr(d, xt_bf, mt):
                """el/er matmuls for all 4 s-subtiles: one [128, SS, 8] psum
                region, then the exp()s on ACT."""
                pcall = mt["pcall"]
                for ss in range(SS):
                    for kc in range(KC):
                        nc.tensor.matmul(pcall[:, ss, :],
                                         lhsT=xt_bf[:, kc, ss * P:(ss + 1) * P],
                                         rhs=cst['wlr_bf'][:, kc, :],
                                         start=(kc == 0), stop=(kc == KC - 1))
                # m-per-sentence: me16 = exp(-0.8*er) straight from psum,
                # written c-major (h outer, ss inner) so the PE transpose
                # sees one contiguous free dim. f32 keeps transpose dtype
                # rules happy (out dtype == lhsT dtype) inside the f32 bank.
                me16 = sp.tile([P, 4 * SS], F32, tag="me16")
                nc.scalar.activation(
                    out=me16,
                    in_=pcall[:, :, 0:4].rearrange("p s c -> p c s"),
                    func=ACTF.Exp, scale=-0.8)
                # a = exp(el), c = exp(0.2 el)  [128, ss, 4] f32 scalars
                a_sc = sp.tile([P, SS, H], F32, tag="asc")
                c_sc = sp.tile([P, SS, H], F32, tag="csc")
                nc.scalar.activation(out=a_sc, in_=pcall[:, :, 4:8], func=ACTF.Exp)
                nc.scalar.activation(out=c_sc, in_=pcall[:, :, 4:8], func=ACTF.Exp,
                                     scale=0.2)
                return me16, a_sc, c_sc

            def stage_mchain(d, me16, mt):
                """er-transpose -> sbuf -> per-row broadcasts (no collapse
                DMA: 16 small Pool broadcasts straight from the 16-partition
                tile, ss-major so the first expe group unblocks after 4)."""
                trm = mt["trm"]
                nc.tensor.transpose(trm, me16, cst['ident_f32'])
                m16sb = sp.tile([4 * SS, P], BF16, tag="m16sb")
                nc.vector.tensor_copy(out=m16sb, in_=trm)
                # collapse to one partition on the Pool software-DGE queue
                # (skips the shared HWDGE + the busy SP queue; broadcast src
                # must start at partition 0, so the collapse is required)
                m_row = sp.tile([1, 4 * SS, P], BF16, tag="mrow")
                nc.gpsimd.dma_start(out=m_row, in_=m16sb[:, None, :])
                m_all = mp.tile([P, 4 * SS, P], BF16, tag="mall")
                for h in range(H):
                    nc.gpsimd.partition_broadcast(
                        m_all[:, h * SS:(h + 1) * SS, :],
                        m_row[:, h * SS:(h + 1) * SS, :])
                return m_all

            def stage_proj_ss(d, xt_bf, ss):
                """projection + h copy for one s-subtile."""
                h_t = hp.tile([P, H, D], BF16, tag=f"ha{ss}")
                pa = ps_proj.tile([P, 512], F32, tag="pab")
                pb = ps_proj.tile([P, 512], F32, tag="pab")
                for kc in range(KC):
                    lt = xt_bf[:, kc, ss * P:(ss + 1) * P]
                    st = (kc == 0)
                    sp_ = (kc == KC - 1)
                    nc.tensor.matmul(pa, lhsT=lt, rhs=cst['w_bf'][:, kc, 0:512],
                                     start=st, stop=sp_)
                    nc.tensor.matmul(pb, lhsT=lt, rhs=cst['w_bf'][:, kc, 512:1024],
                                     start=st, stop=sp_)
                nc.scalar.copy(out=h_t[:, 0:2, :],
                               in_=pa.rearrange("p (h d) -> p h d", h=2))
                nc.scalar.copy(out=h_t[:, 2:4, :],
                               in_=pb.rearrange("p (h d) -> p h d", h=2))
                return h_t

            def stage_expe_g(d, m_all, a_sc, c_sc, eh, ss):
                """expe'[h] = max(a_s, c_s * m_t) for one s-subtile, all heads.
                [128, dst] bf16; 4x DVE mode (all-sbuf, packed bf16)."""
                for h in range(H):
                    nc.vector.tensor_scalar(
                        out=eh[h][:, ss, :],
                        in0=m_all[:, h * SS:(h + 1) * SS, :],
                        scalar1=c_sc[:, ss, h:h + 1],
                        scalar2=a_sc[:, ss, h:h + 1],
                        op0=AX.mult, op1=AX.max)

            def stage_agg_mm(d, ha, eh, mt, dc):
                """aggregation + Z matmuls for one dst chunk."""
                pz = mt["pz"]
                pu01 = ps_agg.tile([P, 2, D], F32, tag="pu")
                pu23 = ps_agg.tile([P, 2, D], F32, tag="pu")
                for h in range(H):
                    pu = (pu01 if h < 2 else pu23)[:, h % 2, :]
                    for sc in range(SS):
                        lt = eh[h][:, sc, dc * P:(dc + 1) * P]
                        nc.tensor.matmul(pu, lhsT=lt, rhs=ha[sc][:, h, :],
                                         start=(sc == 0), stop=(sc == SS - 1))
                        nc.tensor.matmul(pz[:, dc, h:h + 1], lhsT=lt,
                                         rhs=cst['ones4'],
                                         start=(sc == 0), stop=(sc == SS - 1))
                return pu01, pu23

            def stage_combine(d, mt, pu01, pu23, accA, accB, dc):
                """normalize + head-mean for one dst chunk.
                psum reads: 3 fused scale-adds on DVE + 1 scaled copy on ACT
                (gpsimd cannot access PSUM); the final bf16 add on Pool."""
                pz = mt["pz"]
                rz = sp.tile([P, H], F32, tag="rz")
                nc.vector.reciprocal(out=rz, in_=pz[:, dc, :])
                r0 = accp.tile([P, D], F32, tag="r0")
                c2 = accp.tile([P, D], BF16, tag="c2")
                nc.vector.scalar_tensor_tensor(
                    out=r0, in0=pu01[:, 0, :], scalar=rz[:, 0:1],
                    in1=cst['bias_b'], op0=AX.mult, op1=AX.add)
                nc.vector.scalar_tensor_tensor(
                    out=accA[:, dc, :], in0=pu01[:, 1, :], scalar=rz[:, 1:2],
                    in1=r0, op0=AX.mult, op1=AX.add)
                nc.scalar.activation(
                    out=c2, in_=pu23[:, 0, :], func=ACTF.Copy,
                    scale=rz[:, 2:3])
                nc.vector.scalar_tensor_tensor(
                    out=accB[:, dc, :], in0=pu23[:, 1, :], scalar=rz[:, 3:4],
                    in1=c2, op0=AX.mult, op1=AX.add)
                nc.gpsimd.tensor_tensor(out=accA[:, dc, :], in0=accA[:, dc, :],
                                        in1=accB[:, dc, :], op=AX.add)

            def stage_out(d, accA):
                """one DMA for the whole doc's output rows."""
                nc.sync.dma_start(
                    out=out[d * S:(d + 1) * S, :]
                    .rearrange("(dc p) k -> p dc k", p=P),
                    in_=accA)

            # software pipeline: x 2-3 ahead (3-doc prologue); proj(i) is
            # interleaved with agg(i-1) at ss/dc granularity on PE so psum
            # drains overlap matmuls.
            xts = {}
            prev = None  # (d, ha, eh, mt_prev ... ) for agg of doc i-1
            xts[0] = stage_x(0)
            emit_setup()
            for i in range(DPC):
                if i == 0:
                    for j in range(1, min(3, DPC)):
                        xts[j] = stage_x(j)
                elif i + 2 < DPC:
                    xts[i + 2] = stage_x(i + 2)
                xt_bf = xts.pop(i)
                mt = misc_tile()
                eh = [ep.tile([P, SS, S], BF16, tag=f"e{h}", name=f"eh{h}")
                      for h in range(H)]
                ha = [None] * SS

                me16, a_sc, c_sc = stage_elr(i, xt_bf, mt)
                ha[0] = stage_proj_ss(i, xt_bf, 0)
                m_all = stage_mchain(i, me16, mt)
                if prev is not None:
                    accA = accp.tile([P, DC, D], BF16, tag="accA")
                    accB = accp.tile([P, DC, D], BF16, tag="accB")
                # combines for doc i-1 go FIRST in the DVE stream (PE's psum
                # rotation waits on them); the expe groups — gated on the
                # slow m-broadcast chain — go after, so they can't
                # head-of-line-block the combines in DVE's in-order queue.
                for k in range(SS):
                    if k > 0:
                        ha[k] = stage_proj_ss(i, xt_bf, k)
                    if prev is not None:
                        pd, pha, peh, pmt = prev
                        pu01, pu23 = stage_agg_mm(pd, pha, peh, pmt, k)
                        stage_combine(pd, pmt, pu01, pu23, accA, accB, k)
                if prev is not None:
                    stage_out(prev[0], accA)
                for k in range(SS):
                    stage_expe_g(i, m_all, a_sc, c_sc, eh, k)
                prev = (i, ha, eh, mt)

            # drain: agg + combine + out for the last doc
            pd, pha, peh, pmt = prev
            accA = accp.tile([P, DC, D], BF16, tag="accA")
            accB = accp.tile([P, DC, D], BF16, tag="accB")
            for k in range(SS):
                pu01, pu23 = stage_agg_mm(pd, pha, peh, pmt, k)
                stage_combine(pd, pmt, pu01, pu23, accA, accB, k)
            stage_out(pd, accA)


_NC_CACHE = None


def build_nc():
    global _NC_CACHE
    if _NC_CACHE is not None:
        return _NC_CACHE
    nc = bacc.Bacc("TRN2", target_bir_lowering=False, debug=False,
                   num_devices=N_CORES)
    xt = nc.dram_tensor("xt", [DPC, P, KC, S], BF16, kind="ExternalInput")
    w = nc.dram_tensor("w", [K, H * D], BF16, kind="ExternalInput")
    wlr = nc.dram_tensor("wlr", [K, 8], BF16, kind="ExternalInput")
    bias_m = nc.dram_tensor("bias_m", [1, D], BF16, kind="ExternalInput")
    out = nc.dram_tensor("out", [DPC * S, K], BF16, kind="ExternalOutput")
    with tile.TileContext(nc) as tc:
        gat_tile_kernel(tc, xt.ap(), w.ap(), wlr.ap(), bias_m.ap(), out.ap())
    nc.compile()
    _NC_CACHE = nc
    return nc


def kernel(sent_feature, W, attn_l, attn_r, bias, num_docs=NUM_DOCS, **_unused):
    sent_feature = np.asarray(sent_feature, dtype=np.float32)
    W = np.asarray(W, dtype=np.float32)
    attn_l = np.asarray(attn_l, dtype=np.float32)
    attn_r = np.asarray(attn_r, dtype=np.float32)
    bias = np.asarray(bias, dtype=np.float32)

    import ml_dtypes
    bf16 = ml_dtypes.bfloat16
    # host precompute: WLR[k, h] = sum_d W[k, h*D+d]*attn_r[h, d] (cols 0..3)
    # and attn_l (cols 4..7); bias mean over heads; x pre-transposed to
    # [doc, p, kc, s] bf16 (pure layout/dtype prep, per-core sharding).
    w4 = W.reshape(K, H, D)
    wlr = np.concatenate([
        np.einsum("khd,hd->kh", w4, attn_r),
        np.einsum("khd,hd->kh", w4, attn_l),
    ], axis=1).astype(bf16)
    bias_m = bias.reshape(H, D).mean(axis=0, keepdims=True).astype(bf16)
    w_bf = W.astype(bf16)
    xt_full = np.ascontiguousarray(
        sent_feature.reshape(NUM_DOCS, S, KC, P).transpose(0, 3, 2, 1)
    ).astype(bf16)

    nc = build_nc()
    in_maps = []
    for c in range(N_CORES):
        in_maps.append({
            "xt": xt_full[c * DPC:(c + 1) * DPC],
            "w": w_bf, "wlr": wlr, "bias_m": bias_m,
        })
    res = run_bass_kernel_spmd(nc, in_maps, core_ids=list(range(N_CORES)))
    out = np.concatenate([res.results[c]["out"] for c in range(N_CORES)], axis=0)
    return out.astype(np.float32)


# revision 8
# speedup vs baseline: 1.4081x; 1.4081x over previous
"""GAT (DGL GATConv) over complete per-doc graphs — Trainium2 Bass kernel, v3.

Problem: nn_CompletedSentenceGraph (gnn_message_passing).
  64 docs x 512 sentences, HIDDEN=256, HEADS=4, D=256.
  h = (x @ W).reshape(B,S,H,D)
  el/er = einsum(h, attn_l/attn_r)
  e[b,s,t,h] = leaky_relu(el[s]+er[t], 0.2); alpha = softmax over s
  out = einsum(alpha, h) + bias; return mean over heads  -> [N, 256]

Sharding: data-parallel over docs, 8 docs per core on 8 cores.

Math tricks (same family as v2):
  * exp(lrelu(x)) = max(exp(x), exp(0.2 x)); with x = el_s + er_t both exps
    are rank-1: expe'[s,t] = max(a_s, c_s * m_t), a=exp(el), c=exp(0.2 el),
    m=exp(-0.8 er) (per-dst scaling; softmax-invariant).
  * el/er from an augmented tiny matmul (WLR = W @ ALR built on host).
  * Z (softmax denominator) via near-free N=1 matmuls with a 4.0-ones rhs;
    4.0 folds the 1/H head-mean.

v3 pipeline changes (driven by TimelineSim engine-occupancy analysis):
  * One shared PSUM bank ("misc") holds the elr results + the er-transpose
    + the Z columns (PSUM slots are bank-padded, so separate tags would
    burn 3 of 8 banks); the freed bank deepens the agg pool to 4 banks.
  * proj ss-chunks and agg dc-chunks interleave on PE so psum drains
    (ACT/DVE) overlap matmuls instead of stalling them.
  * Head-combine rebalanced: DVE does rz + 3 fused scale-adds, ACT does 1
    scaled copy, the final bf16 add runs on the idle Pool engine.
  * 1 output DMA per doc (was 4); the m-row collapse DMA moved to the
    Pool software-DGE queue (off the shared HWDGE + SP queue).
  * Startup DMA order: xt(0), wlr, w, bias, xt(1), xt(2) — gets the first
    elr/proj matmuls their operands ~1us sooner.
"""

from contextlib import ExitStack

import numpy as np

import concourse.mybir as mybir
import concourse.tile as tile
from concourse import bacc
from concourse.bass_utils import run_bass_kernel_spmd
from concourse.masks import make_identity

F32 = mybir.dt.float32
BF16 = mybir.dt.bfloat16
AX = mybir.AluOpType
ACTF = mybir.ActivationFunctionType

NUM_DOCS = 64
S = 512          # sentences per doc
K = 256          # hidden
H = 4            # heads
D = 256          # per-head out feats
N_CORES = 8
DPC = NUM_DOCS // N_CORES  # docs per core
P = 128

SS = S // P      # 4 s-subtiles per doc
KC = K // P      # 2 k-chunks
DC = S // P      # 4 dst chunks

# misc psum bank layout (f32 columns)
MC_ELR = 0       # [P, SS, 8] elr results: cols 0..32
MC_TRM = 32      # [16, 128] er-transpose: cols 32..160 (partitions 0..16)
MC_PZ = 160      # [P, DC, H] Z sums: cols 160..176


def gat_tile_kernel(tc, xt, w, wlr, bias_m, out):
    """xt [DPC, 128, KC, 512] bf16 = host-pre-transposed x;
    w [256, 1024] bf16; wlr [256, 8] bf16 = host [W@attn_r | W@attn_l];
    bias_m [1, 256] bf16 = host mean-over-heads bias."""
    nc = tc.nc

    stack = ExitStack()
    with stack:
        consts = stack.enter_context(tc.tile_pool(name="consts", bufs=1))
        ps_misc = stack.enter_context(
            tc.tile_pool(name="ps_misc", bufs=1, space="PSUM"))
        # setup consts are emitted AFTER the first x load (see below) so the
        # doc-0 x DMA gets the head of the SP queue; setup_tmp stays open for
        # the whole program: closing it would emit a pool-exit barrier that
        # stalls the SP queue.
        setup_tmp = stack.enter_context(tc.tile_pool(name="setup_tmp", bufs=1))
        cst = {}

        def emit_setup():
            ident_f32 = consts.tile([P, P], F32)
            make_identity(nc, ident_f32)

            # weights arrive bf16 from the host: plain DMAs, no converts.
            # wlr FIRST (elr needs it before proj needs w).
            wlr_bf = consts.tile([P, KC, 8], BF16)
            nc.sync.dma_start(out=wlr_bf,
                              in_=wlr.rearrange("(kc p) c -> p kc c", p=P))
            # w split by k-chunk so the first proj matmuls (which only need
            # kc0) unblock one transfer earlier
            w_bf = consts.tile([P, KC, H * D], BF16)
            w_re = w.rearrange("(kc p) f -> p kc f", p=P)
            for kc in range(KC):
                nc.sync.dma_start(out=w_bf[:, kc], in_=w_re[:, kc])
            # bias is not needed until the first combine (>1 doc later)
            bias_mf = setup_tmp.tile([1, D], BF16)
            nc.gpsimd.dma_start(out=bias_mf, in_=bias_m)
            bias_b = consts.tile([P, D], BF16)
            nc.gpsimd.partition_broadcast(bias_b, bias_mf)

            # ones column for the Z matmuls; 4.0 folds the 1/H head mean
            ones4 = consts.tile([P, 1], BF16)
            nc.gpsimd.memset(ones4, 4.0)
            cst.update(ident_f32=ident_f32, w_bf=w_bf,
                       wlr_bf=wlr_bf, bias_b=bias_b, ones4=ones4)

        # ---------------- per-doc pipeline ----------------
        with tc.tile_pool(name="xtp", bufs=4) as xtp, \
             tc.tile_pool(name="hp", bufs=3) as hp, \
             tc.tile_pool(name="ep", bufs=3) as ep, \
             tc.tile_pool(name="mp", bufs=3) as mp, \
             tc.tile_pool(name="sp", bufs=4) as sp, \
             tc.tile_pool(name="accp", bufs=3) as accp, \
             tc.tile_pool(name="ps_proj", bufs=3, space="PSUM") as ps_proj, \
             tc.tile_pool(name="ps_agg", bufs=4, space="PSUM") as ps_agg:

            def stage_x(d):
                """xT arrives pre-transposed/pre-bf16 from the host: 1 DMA."""
                xt_bf = xtp.tile([P, KC, S], BF16)
                nc.sync.dma_start(out=xt_bf, in_=xt[d])
                return xt_bf

            # One LONG-LIVED psum bank shared by elr results / transpose / Z.
            # A single tile object (not a per-doc ring allocation) so the
            # framework tracks the three regions at subtile granularity:
            # doc i's elr only waits on doc i-1's reads of the SAME region,
            # not on the Z reads that are still in flight for doc i-1.
            misc = ps_misc.tile([P, 512], F32, tag="misc", name="misc")
            mt_shared = {
                "pcall": misc[:, MC_ELR:MC_ELR + SS * 8]
                .rearrange("p (s c) -> p s c", s=SS),
                "trm": misc[0:4 * SS, MC_TRM:MC_TRM + P],
                "pz": misc[:, MC_PZ:MC_PZ + DC * H]
                .rearrange("p (d h) -> p d h", d=DC),
            }

            def misc_tile():
                return mt_shared

            def stage_elr(d, xt_bf, mt):
                """el/er matmuls for all 4 s-subtiles: one [128, SS, 8] psum
                region, then the exp()s on ACT."""
                pcall = mt["pcall"]
                for ss in range(SS):
                    for kc in range(KC):
                        nc.tensor.matmul(pcall[:, ss, :],
                                         lhsT=xt_bf[:, kc, ss * P:(ss + 1) * P],
                                         rhs=cst['wlr_bf'][:, kc, :],
                                         start=(kc == 0), stop=(kc == KC - 1))
                # m-per-sentence: me16 = exp(-0.8*er) straight from psum,
                # written c-major (h outer, ss inner) so the PE transpose
                # sees one contiguous free dim. f32 keeps transpose dtype
                # rules happy (out dtype == lhsT dtype) inside the f32 bank.
                me16 = sp.tile([P, 4 * SS], F32, tag="me16")
                nc.scalar.activation(
                    out=me16,
                    in_=pcall[:, :, 0:4].rearrange("p s c -> p c s"),
                    func=ACTF.Exp, scale=-0.8)
                # a = exp(el), c = exp(0.2 el)  [128, ss, 4] f32 scalars
                a_sc = sp.tile([P, SS, H], F32, tag="asc")
                c_sc = sp.tile([P, SS, H], F32, tag="csc")
                nc.scalar.activation(out=a_sc, in_=pcall[:, :, 4:8], func=ACTF.Exp)
                nc.scalar.activation(out=c_sc, in_=pcall[:, :, 4:8], func=ACTF.Exp,
                                     scale=0.2)
                return me16, a_sc, c_sc

            def stage_mchain(d, me16, mt):
                """er-transpose -> sbuf -> per-row broadcasts (no collapse
                DMA: 16 small Pool broadcasts straight from the 16-partition
                tile, ss-major so the first expe group unblocks after 4)."""
                trm = mt["trm"]
                nc.tensor.transpose(trm, me16, cst['ident_f32'])
                m16sb = sp.tile([4 * SS, P], BF16, tag="m16sb")
                nc.vector.tensor_copy(out=m16sb, in_=trm)
                # collapse to one partition on the Pool software-DGE queue
                # (skips the shared HWDGE + the busy SP queue; broadcast src
                # must start at partition 0, so the collapse is required)
                m_row = sp.tile([1, 4 * SS, P], BF16, tag="mrow")
                nc.gpsimd.dma_start(out=m_row, in_=m16sb[:, None, :])
                m_all = mp.tile([P, 4 * SS, P], BF16, tag="mall")
                for h in range(H):
                    nc.gpsimd.partition_broadcast(
                        m_all[:, h * SS:(h + 1) * SS, :],
                        m_row[:, h * SS:(h + 1) * SS, :])
                return m_all

            def stage_proj_ss(d, xt_bf, ss):
                """projection + h copy for one s-subtile."""
                h_t = hp.tile([P, H, D], BF16, tag=f"ha{ss}")
                pa = ps_proj.tile([P, 512], F32, tag="pab")
                pb = ps_proj.tile([P, 512], F32, tag="pab")
                for kc in range(KC):
                    lt = xt_bf[:, kc, ss * P:(ss + 1) * P]
                    st = (kc == 0)
                    sp_ = (kc == KC - 1)
                    nc.tensor.matmul(pa, lhsT=lt, rhs=cst['w_bf'][:, kc, 0:512],
                                     start=st, stop=sp_)
                    nc.tensor.matmul(pb, lhsT=lt, rhs=cst['w_bf'][:, kc, 512:1024],
                                     start=st, stop=sp_)
                nc.scalar.copy(out=h_t[:, 0:2, :],
                               in_=pa.rearrange("p (h d) -> p h d", h=2))
                nc.scalar.copy(out=h_t[:, 2:4, :],
                               in_=pb.rearrange("p (h d) -> p h d", h=2))
                return h_t

            def stage_expe_g(d, m_all, a_sc, c_sc, eh, ss):
                """expe'[h] = max(a_s, c_s * m_t) for one s-subtile, all heads.
                [128, dst] bf16; 4x DVE mode (all-sbuf, packed bf16)."""
                for h in range(H):
                    nc.vector.tensor_scalar(
                        out=eh[h][:, ss, :],
                        in0=m_all[:, h * SS:(h + 1) * SS, :],
                        scalar1=c_sc[:, ss, h:h + 1],
                        scalar2=a_sc[:, ss, h:h + 1],
                        op0=AX.mult, op1=AX.max)

            def stage_agg_mm(d, ha, eh, mt, dc):
                """aggregation + Z matmuls for one dst chunk."""
                pz = mt["pz"]
                pu01 = ps_agg.tile([P, 2, D], F32, tag="pu")
                pu23 = ps_agg.tile([P, 2, D], F32, tag="pu")
                for h in range(H):
                    pu = (pu01 if h < 2 else pu23)[:, h % 2, :]
                    for sc in range(SS):
                        lt = eh[h][:, sc, dc * P:(dc + 1) * P]
                        nc.tensor.matmul(pu, lhsT=lt, rhs=ha[sc][:, h, :],
                                         start=(sc == 0), stop=(sc == SS - 1))
                        nc.tensor.matmul(pz[:, dc, h:h + 1], lhsT=lt,
                                         rhs=cst['ones4'],
                                         start=(sc == 0), stop=(sc == SS - 1))
                return pu01, pu23

            def stage_combine(d, mt, pu01, pu23, accA, accB, dc):
                """normalize + head-mean for one dst chunk.
                psum reads: 3 fused scale-adds on DVE + 1 scaled copy on ACT
                (gpsimd cannot access PSUM); the final bf16 add on Pool."""
                pz = mt["pz"]
                rz = sp.tile([P, H], F32, tag="rz")
                nc.vector.reciprocal(out=rz, in_=pz[:, dc, :])
                r0 = accp.tile([P, D], F32, tag="r0")
                c2 = accp.tile([P, D], BF16, tag="c2")
                nc.vector.scalar_tensor_tensor(
                    out=r0, in0=pu01[:, 0, :], scalar=rz[:, 0:1],
                    in1=cst['bias_b'], op0=AX.mult, op1=AX.add)
                nc.vector.scalar_tensor_tensor(
                    out=accA[:, dc, :], in0=pu01[:, 1, :], scalar=rz[:, 1:2],
                    in1=r0, op0=AX.mult, op1=AX.add)
                nc.scalar.activation(
                    out=c2, in_=pu23[:, 0, :], func=ACTF.Copy,
                    scale=rz[:, 2:3])
                nc.vector.scalar_tensor_tensor(
                    out=accB[:, dc, :], in0=pu23[:, 1, :], scalar=rz[:, 3:4],
                    in1=c2, op0=AX.mult, op1=AX.add)
                nc.gpsimd.tensor_tensor(out=accA[:, dc, :], in0=accA[:, dc, :],
                                        in1=accB[:, dc, :], op=AX.add)

            def stage_out(d, accA):
                """one DMA for the whole doc's output rows."""
                nc.sync.dma_start(
                    out=out[d * S:(d + 1) * S, :]
                    .rearrange("(dc p) k -> p dc k", p=P),
                    in_=accA)

            # software pipeline: x 2-3 ahead (3-doc prologue); proj(i) is
            # interleaved with agg(i-1) at ss/dc granularity on PE so psum
            # drains overlap matmuls.
            xts = {}
            prev = None  # (d, ha, eh, mt_prev ... ) for agg of doc i-1
            xts[0] = stage_x(0)
            emit_setup()
            for i in range(DPC):
                if i == 0:
                    for j in range(1, min(3, DPC)):
                        xts[j] = stage_x(j)
                elif i + 2 < DPC:
                    xts[i + 2] = stage_x(i + 2)
                xt_bf = xts.pop(i)
                mt = misc_tile()
                eh = [ep.tile([P, SS, S], BF16, tag=f"e{h}", name=f"eh{h}")
                      for h in range(H)]
                ha = [None] * SS

                me16, a_sc, c_sc = stage_elr(i, xt_bf, mt)
                ha[0] = stage_proj_ss(i, xt_bf, 0)
                m_all = stage_mchain(i, me16, mt)
                if prev is not None:
                    accA = accp.tile([P, DC, D], BF16, tag="accA")
                    accB = accp.tile([P, DC, D], BF16, tag="accB")
                # combines for doc i-1 go FIRST in the DVE stream (PE's psum
                # rotation waits on them); the expe groups — gated on the
                # slow m-broadcast chain — go after, so they can't
                # head-of-line-block the combines in DVE's in-order queue.
                for k in range(SS):
                    if k > 0:
                        ha[k] = stage_proj_ss(i, xt_bf, k)
                    if prev is not None:
                        pd, pha, peh, pmt = prev
                        pu01, pu23 = stage_agg_mm(pd, pha, peh, pmt, k)
                        stage_combine(pd, pmt, pu01, pu23, accA, accB, k)
                if prev is not None:
                    stage_out(prev[0], accA)
                for k in range(SS):
                    stage_expe_g(i, m_all, a_sc, c_sc, eh, k)
                prev = (i, ha, eh, mt)

            # drain: agg + combine + out for the last doc
            pd, pha, peh, pmt = prev
            accA = accp.tile([P, DC, D], BF16, tag="accA")
            accB = accp.tile([P, DC, D], BF16, tag="accB")
            for k in range(SS):
                pu01, pu23 = stage_agg_mm(pd, pha, peh, pmt, k)
                stage_combine(pd, pmt, pu01, pu23, accA, accB, k)
            stage_out(pd, accA)


_NC_CACHE = None


def build_nc():
    global _NC_CACHE
    if _NC_CACHE is not None:
        return _NC_CACHE
    nc = bacc.Bacc("TRN2", target_bir_lowering=False, debug=False,
                   num_devices=N_CORES)
    xt = nc.dram_tensor("xt", [DPC, P, KC, S], BF16, kind="ExternalInput")
    w = nc.dram_tensor("w", [K, H * D], BF16, kind="ExternalInput")
    wlr = nc.dram_tensor("wlr", [K, 8], BF16, kind="ExternalInput")
    bias_m = nc.dram_tensor("bias_m", [1, D], BF16, kind="ExternalInput")
    out = nc.dram_tensor("out", [DPC * S, K], BF16, kind="ExternalOutput")
    with tile.TileContext(nc) as tc:
        gat_tile_kernel(tc, xt.ap(), w.ap(), wlr.ap(), bias_m.ap(), out.ap())
    nc.compile()
    _NC_CACHE = nc
    return nc


def kernel(sent_feature, W, attn_l, attn_r, bias, num_docs=NUM_DOCS, **_unused):
    sent_feature = np.asarray(sent_feature, dtype=np.float32)
    W = np.asarray(W, dtype=np.float32)
    attn_l = np.asarray(attn_l, dtype=np.float32)
    attn_r = np.asarray(attn_r, dtype=np.float32)
    bias = np.asarray(bias, dtype=np.float32)

    import ml_dtypes
    bf16 = ml_dtypes.bfloat16
    # host precompute: WLR[k, h] = sum_d W[k, h*D+d]*attn_r[h, d] (cols 0..3)
    # and attn_l (cols 4..7); bias mean over heads; x pre-transposed to
    # [doc, p, kc, s] bf16 (pure layout/dtype prep, per-core sharding).
    w4 = W.reshape(K, H, D)
    wlr = np.concatenate([
        np.einsum("khd,hd->kh", w4, attn_r),
        np.einsum("khd,hd->kh", w4, attn_l),
    ], axis=1).astype(bf16)
    bias_m = bias.reshape(H, D).mean(axis=0, keepdims=True).astype(bf16)
    w_bf = W.astype(bf16)
    xt_full = np.ascontiguousarray(
        sent_feature.reshape(NUM_DOCS, S, KC, P).transpose(0, 3, 2, 1)
    ).astype(bf16)

    nc = build_nc()
    in_maps = []
    for c in range(N_CORES):
        in_maps.append({
            "xt": xt_full[c * DPC:(c + 1) * DPC],
            "w": w_bf, "wlr": wlr, "bias_m": bias_m,
        })
    res = run_bass_kernel_spmd(nc, in_maps, core_ids=list(range(N_CORES)))
    out = np.concatenate([res.results[c]["out"] for c in range(N_CORES)], axis=0)
    return out.astype(np.float32)
